# revision 1
# baseline (speedup 1.0000x reference)
import sys
sys.path.insert(0, '/opt/trn_rl_repo')
import numpy as np
import concourse.bass as bass
import concourse.tile as tile
from concourse import bacc, mybir
from contextlib import ExitStack

F32 = mybir.dt.float32
BF16 = mybir.dt.bfloat16
I16 = mybir.dt.int16
I32 = mybir.dt.int32

H = W = 80
HW = H * W
C = 128
OUTC = 256
NPT = 9
NTOK = 6784
RANKS = NTOK // 128   # 53
CHUNK = 640
NCHUNK = HW // CHUNK  # 10
CPC = CHUNK // 16     # 40
EPS = 1e-5

mult = mybir.AluOpType.mult
add_ = mybir.AluOpType.add
sub_ = mybir.AluOpType.subtract
AF = mybir.ActivationFunctionType


def build(nc, dbg=()):
    dbg = set(dbg)
    x_ap = nc.dram_tensor("x", [C, HW], F32, kind="ExternalInput").ap()
    w_offT_ap = nc.dram_tensor("w_offT", [1152, 18], F32, kind="ExternalInput").ap()
    b_off_ap = nc.dram_tensor("b_off", [18, 1], F32, kind="ExternalInput").ap()
    w_convT_ap = nc.dram_tensor("w_convT", [1152, OUTC], F32, kind="ExternalInput").ap()
    bn_ap = nc.dram_tensor("bn", [128, 8], F32, kind="ExternalInput").ap()
    out_ap = nc.dram_tensor("out", [OUTC, HW], F32, kind="ExternalOutput").ap()
    off_dram = nc.dram_tensor("off_dram", [18, HW], F32).ap()   # internal bounce
    patch_dram = nc.dram_tensor("patch_dram", [NTOK, 4 * C], BF16).ap()
    dbg_aps = {}
    def dbg_out(name, shape, dt):
        if name in dbg:
            dbg_aps[name] = nc.dram_tensor("dbg_" + name, shape, dt, kind="ExternalOutput").ap()
            return dbg_aps[name]
        return None

    with tile.TileContext(nc) as tc, ExitStack() as ctx:
        con = ctx.enter_context(tc.tile_pool(name="const", bufs=1))
        big = ctx.enter_context(tc.tile_pool(name="bigb", bufs=1))
        tmp = ctx.enter_context(tc.tile_pool(name="tmp", bufs=2))
        scr = ctx.enter_context(tc.tile_pool(name="scr", bufs=1))
        psA = ctx.enter_context(tc.tile_pool(name="psA", bufs=1, space="PSUM"))
        psW = ctx.enter_context(tc.tile_pool(name="psW", bufs=1, space="PSUM"))

        # ---------- weights / params ----------
        w_off_t = []
        for t in range(9):
            wf = tmp.tile([C, 18], F32, tag="wofl", name="wofl")
            nc.sync.dma_start(wf[:], w_offT_ap[t*128:(t+1)*128, :])
            wb = con.tile([C, 18], BF16, tag=f"woff{t}", name=f"woff{t}")
            nc.vector.tensor_copy(wb[:], wf[:])
            w_off_t.append(wb)
        w_conv_t = []
        for t in range(9):
            wf = tmp.tile([C, OUTC], F32, tag="wcvl", name="wcvl")
            nc.sync.dma_start(wf[:], w_convT_ap[t*128:(t+1)*128, :])
            wb = con.tile([C, OUTC], BF16, tag=f"wconv{t}", name=f"wconv{t}")
            nc.vector.tensor_copy(wb[:], wf[:])
            w_conv_t.append(wb)
        b_off = con.tile([18, 1], F32)
        nc.sync.dma_start(b_off[:], b_off_ap[:, :])
        bn = con.tile([128, 8], F32)
        nc.sync.dma_start(bn[:], bn_ap[:, :])
        bnv = con.tile([128, 2], F32, tag="bnv", name="bnv")
        nc.vector.tensor_scalar(bnv[:], bn[:, 6:8], EPS, None, op0=add_)
        bnsq = con.tile([128, 2], F32, tag="bnsq", name="bnsq")
        nc.scalar.activation(bnsq[:], bnv[:], AF.Sqrt)
        bnrs = con.tile([128, 2], F32, tag="bnrs", name="bnrs")
        nc.vector.reciprocal(bnrs[:], bnsq[:])
        bnscale = con.tile([128, 2], F32, tag="bnscale", name="bnscale")
        nc.vector.tensor_tensor(bnscale[:], bn[:, 0:2], bnrs[:], op=mult)
        bnmus = con.tile([128, 2], F32, tag="bnmus", name="bnmus")
        nc.vector.tensor_tensor(bnmus[:], bn[:, 4:6], bnscale[:], op=mult)
        bnbias = con.tile([128, 2], F32, tag="bnbias", name="bnbias")
        nc.vector.tensor_tensor(bnbias[:], bn[:, 2:4], bnmus[:], op=sub_)
        ones1 = con.tile([1, 128], BF16, tag="ones1", name="ones1")
        nc.vector.memset(ones1[:], 1.0)
        ids32 = tmp.tile([128, 128], I32, tag="ids32", name="ids32")
        nc.gpsimd.iota(ids32[:], [[1, 128]], base=0, channel_multiplier=0)
        idp32 = tmp.tile([128, 128], I32, tag="idp32", name="idp32")
        nc.gpsimd.iota(idp32[:], [[0, 128]], base=0, channel_multiplier=1)
        idf1 = tmp.tile([128, 128], F32, tag="idf1", name="idf1")
        nc.vector.tensor_copy(idf1[:], ids32[:])
        idf2 = tmp.tile([128, 128], F32, tag="idf2", name="idf2")
        nc.vector.tensor_copy(idf2[:], idp32[:])
        ident = con.tile([128, 128], BF16, tag="ident", name="ident")
        nc.vector.tensor_tensor(ident[:], idf1[:], idf2[:], op=mybir.AluOpType.is_equal)

        # ---------- P1: xb (bf16), xpad ----------
        xb = big.tile([C, HW], BF16)
        for q in range(4):
            xfq = tmp.tile([C, 1600], F32, tag="xfq", name="xfq", bufs=1)
            nc.sync.dma_start(xfq[:], x_ap[:, q*1600:(q+1)*1600])
            nc.vector.tensor_copy(xb[:, q*1600:(q+1)*1600], xfq[:])
        xpad = scr.tile([C, 82 * 82], BF16, tag="ph13", name="xpad")
        nc.vector.memset(xpad[:], 0.0)
        xpv = xpad[:, :].rearrange("p (a b) -> p a b", b=82)
        nc.sync.dma_start(xpv[:, 1:81, 1:81],
                          xb[:, :].rearrange("p (a b) -> p a b", b=80))

        # ---------- P2: offset conv -> off_dram ----------
        row_tiles = [(0, 12), (12, 12), (24, 12), (36, 12), (48, 12), (60, 12), (72, 8)]
        for rix, (r0, nr) in enumerate(row_tiles):
            nb = (nr + 5) // 6   # 6-row (480 px) sub-blocks
            ps = psW.tile([18, 2, 512], F32, tag="wps", name="offps", bufs=2)
            for b in range(nb):
                sr0, snr = r0 + b*6, min(6, r0 + nr - (r0 + b*6))
                for t in range(9):
                    di, dj = t // 3, t % 3
                    src = xpad[:, :].rearrange("p (a b) -> p a b", b=82)[
                        :, sr0 + di: sr0 + di + snr, dj: dj + 80]
                    nc.tensor.matmul(ps[:, b, 0:snr*80], w_off_t[t][:], src,
                                     start=(t == 0), stop=(t == 8))
            ost = tmp.tile([18, 2, 480], F32, tag="ost", name="ost", bufs=2)
            for b in range(nb):
                snr = min(6, nr - b*6)
                nc.scalar.activation(ost[:, b, 0:snr*80], ps[:, b, 0:snr*80],
                                     AF.Identity, bias=b_off[:, 0:1], scale=1.0)
            if nr == 12:
                nc.sync.dma_start(off_dram[:, r0*80:(r0+nr)*80], ost[:, 0:nb, :])
            else:
                nc.sync.dma_start(off_dram[:, r0*80:(r0+nr)*80],
                                  ost[:, 0:nb, :].rearrange("p a b -> p (a b)")[:, 0:nr*80])
        if (a := dbg_out("off", [18, HW], F32)) is not None:
            nc.sync.dma_start(a[:, :], off_dram[:, :])

        # ---------- P3: offsets -> (80, 9, 80) ----------
        offx = scr.tile([80, NPT, 80], F32, tag="offx", name="offx")
        offy = scr.tile([80, NPT, 80], F32, tag="offy", name="offy")
        for k in range(NPT):
            nc.sync.dma_start(offx[:, k, :], off_dram[k, :].rearrange("(a b) -> a b", b=80))
            nc.scalar.dma_start(offy[:, k, :], off_dram[NPT+k, :].rearrange("(a b) -> a b", b=80))

        # ---------- P4: index & weight math ----------
        shape = [80, NPT, 80]
        S = [scr.tile(shape, F32, tag=f"s{i}", name=f"s{i}") for i in range(10)]
        Si = scr.tile(shape, I32, tag="si", name="si")

        def iota_into(dst, pattern, base, chmul):
            nc.gpsimd.iota(Si[:], pattern, base=base, channel_multiplier=chmul)
            nc.vector.tensor_copy(dst[:], Si[:])

        def floor_frac(io_t, offm, bout, dout, t1, t2):
            nc.vector.tensor_tensor(t1[:], io_t[:], offm[:], op=add_)
            nc.vector.tensor_copy(Si[:], t1[:])
            nc.vector.tensor_copy(t2[:], Si[:])
            nc.vector.tensor_tensor(bout[:], t2[:], t1[:], op=mybir.AluOpType.is_gt)
            nc.vector.tensor_tensor(bout[:], t2[:], bout[:], op=sub_)
            nc.vector.tensor_tensor(dout[:], t1[:], bout[:], op=sub_)

        def valid_into(vout, b, lo, hi, t1, t2):
            nc.vector.tensor_scalar(t1[:], b[:], float(lo), None, op0=mybir.AluOpType.is_ge)
            nc.vector.tensor_scalar(t2[:], b[:], float(hi), None, op0=mybir.AluOpType.is_le)
            nc.vector.tensor_tensor(vout[:], t1[:], t2[:], op=mult)

        iota_into(S[0], [[1, 3], [0, 3], [0, 80]], 3, 1)     # i + pr + 4
        floor_frac(S[0], offx, S[1], S[2], S[3], S[4])       # S1=x0f S2=dx
        valid_into(S[5], S[1], 4, 83, S[3], S[4])            # vr0
        valid_into(S[6], S[1], 3, 82, S[3], S[4])            # vr1
        wx0, wx1 = S[7], S[8]
        nc.vector.tensor_scalar(S[3][:], S[2][:], -1.0, 1.0, op0=mult, op1=add_)
        nc.vector.tensor_tensor(wx0[:], S[3][:], S[5][:], op=mult)
        nc.vector.tensor_tensor(wx1[:], S[2][:], S[6][:], op=mult)
        nc.vector.tensor_scalar(S[3][:], S[1][:], 3.0, None, op0=mybir.AluOpType.max)
        nc.vector.tensor_scalar(S[9][:], S[3][:], 83.0, -4.0, op0=mybir.AluOpType.min, op1=add_)
        brow = S[9]
        iota_into(S[0], [[0, 3], [1, 3], [1, 80]], 3, 0)     # j + pc + 4
        floor_frac(S[0], offy, S[1], S[2], S[3], S[4])       # S1=y0f S2=dy
        valid_into(S[5], S[1], 4, 83, S[3], S[4])            # vc0
        valid_into(S[6], S[1], 3, 82, S[3], S[4])            # vc1
        wy0, wy1 = S[0], S[4]
        nc.vector.tensor_scalar(S[3][:], S[2][:], -1.0, 1.0, op0=mult, op1=add_)
        nc.vector.tensor_tensor(wy0[:], S[3][:], S[5][:], op=mult)
        nc.vector.tensor_tensor(wy1[:], S[2][:], S[6][:], op=mult)
        nc.vector.tensor_scalar(S[2][:], S[1][:], 3.0, None, op0=mybir.AluOpType.max)
        nc.vector.tensor_scalar(S[3][:], S[2][:], 83.0, -4.0, op0=mybir.AluOpType.min, op1=add_)
        nc.vector.tensor_scalar(S[5][:], brow[:], 80.0, 128.0, op0=mult, op1=add_)
        nc.vector.tensor_tensor(S[5][:], S[5][:], S[3][:], op=add_)
        idx16 = con.tile(shape, I16, tag="idx16", name="idx16")
        nc.vector.tensor_copy(idx16[:], S[5][:])
        if (a := dbg_out("idx", [80, NPT * 80], F32)) is not None:
            nc.sync.dma_start(a[:, :], S[5][:, :, :])
        w4all = con.tile([36, HW], BF16, tag="w4all", name="w4all")
        # one-hot row selectors for the broadcast matmul (contraction over 36)
        selp32 = tmp.tile([36, 128], I32, tag="selp32", name="selp32")
        nc.gpsimd.iota(selp32[:], [[0, 128]], base=0, channel_multiplier=1)
        selpf = tmp.tile([36, 128], F32, tag="selpf", name="selpf")
        nc.vector.tensor_copy(selpf[:], selp32[:])
        sels = []
        for r in range(36):
            sf = tmp.tile([36, 128], F32, tag="self", name="self")
            nc.vector.tensor_scalar(sf[:], selpf[:], float(r), None, op0=mybir.AluOpType.is_equal)
            sb = con.tile([36, 128], BF16, tag=f"sel{r}", name=f"sel{r}")
            nc.vector.tensor_copy(sb[:], sf[:])
            sels.append(sb)
        wqf = S[6]
        wqb = con.tile(shape, BF16, tag="wqb", name="wqb")
        for j, (a_, b_) in enumerate([(wx0, wy0), (wx0, wy1), (wx1, wy0), (wx1, wy1)]):
            nc.vector.tensor_tensor(wqf[:], a_[:], b_[:], op=mult)
            nc.vector.tensor_copy(wqb[:], wqf[:])
            if (ad := dbg_out(f"wq{j}", [80, NPT * 80], BF16)) is not None:
                nc.sync.dma_start(ad[:, :], wqb[:, :, :])
            for k in range(NPT):
                eng = nc.sync if k % 2 == 0 else nc.scalar
                eng.dma_start(
                    w4all[k*4+j:k*4+j+1, :].rearrange("p (a b) -> p a b", b=80),
                    wqb[:, k, :])

        # ---------- P5a: idx wrap (128, 9, 400), log-replicated ----------
        iw9 = con.tile([128, NPT, HW // 16], I16, tag="iw9", name="iw9")
        for k in range(NPT):
            eng = nc.sync if k % 2 == 0 else nc.scalar
            eng.dma_start(iw9[0:16, k, :], idx16[:, k, :])
        nc.sync.dma_start(iw9[16:32, :, :], iw9[0:16, :, :])
        nc.sync.dma_start(iw9[32:64, :, :], iw9[0:32, :, :])
        nc.sync.dma_start(iw9[64:128, :, :], iw9[0:64, :, :])

        # ---------- P5b: xTp + patch ----------
        xTp = scr.tile([128, RANKS, C], BF16, tag="ph13", name="xTp")
        nc.vector.memset(xTp[:, 0, :], 0.0)
        nc.vector.memset(xTp[:, 51, :], 0.0)
        nc.vector.memset(xTp[:, 52, :], 0.0)
        for q4 in range(13):   # 50 tiles in 13 rounds of <=4
            rr = list(range(1 + q4*4, min(1 + q4*4 + 4, 51)))
            pst = psA.tile([128, 4, 128], BF16, tag=("y10" if q4 % 2 == 0 else "y11"),
                           name="pst")
            for bi, r in enumerate(rr):
                nc.tensor.transpose(pst[:, bi, :], xb[:, (r-1)*128:r*128], ident[:])
            nc.scalar.activation(xTp[:, rr[0]:rr[0]+len(rr), :], pst[:, 0:len(rr), :],
                                 AF.Copy)
        DELTA = [0, 1, 80, 81]
        # DRAM patch: patch_dram[u, j, :] = xTp token (u + delta_j)
        pdv = patch_dram.rearrange("u (j c) -> u j c", c=C)
        ztail = tmp.tile([128, 4 * C], BF16, tag="ztail", name="ztail", bufs=1)
        nc.vector.memset(ztail[:], 0.0)
        nc.sync.dma_start(patch_dram[NTOK-128:NTOK, :], ztail[:, :])
        for j, d in enumerate(DELTA):
            lo = d % 128
            if lo == 0:
                # dst token u = 128 r + p  <- src (p, r)
                dst = pdv[:, j, :].rearrange("(r p) c -> p r c", p=128)
                nc.sync.dma_start(dst, xTp[:, :, :])
            else:
                # u in [0, NTOK-128): dst token u = 128 r + p <- src partition p+lo
                dstA = pdv[0:NTOK-128, j, :].rearrange("(r p) c -> p r c", p=128)
                nc.sync.dma_start(dstA[0:128-lo, :, :], xTp[lo:128, 0:RANKS-1, :])
                nc.sync.dma_start(dstA[128-lo:128, :, :], xTp[0:lo, 1:RANKS, :])
                # tokens >= NTOK-128+... never addressed (max 6608)
        if (a := dbg_out("patch", [NTOK, 4 * C], BF16)) is not None:
            nc.sync.dma_start(a[:, :], patch_dram[:, :])

        # ---------- P6/P7: chunks ----------
        gpool = ctx.enter_context(tc.tile_pool(name="gp", bufs=3))
        g_ctr = 0
        w4vc = w4all[:, :].rearrange("p (q c) -> p c q", q=16)  # (36, 400, 16)
        chunk_list = [(i * 56, 56) for i in range(7)] + [(392, 8)]
        for cix, (c0, cpc) in enumerate(chunk_list):
            CH = cpc * 16            # positions in this chunk
            NH = CH // 2             # psum subtile width
            yps = [[psA.tile([128, 448], F32, tag=f"y{ot}{nt}", name=f"y{ot}{nt}")
                    for nt in range(2)] for ot in range(2)]
            for k in range(NPT):
                g = gpool.tile([128, 4, CH], BF16, tag=f"g{g_ctr % 2}", name="g", bufs=3)[:, :, :]
                nc.gpsimd.dma_gather(
                    out_ap=g, in_ap=patch_dram,
                    idxs_ap=iw9[:, k, c0:c0 + cpc],
                    num_idxs=CH, num_idxs_reg=CH,
                    elem_size=4 * C, transpose=True,
                    queue_num=g_ctr % nc.num_swdge_queues,
                )
                g_ctr += 1
                w4r = gpool.tile([128, 4, CH], BF16, tag="w4r", name="w4r", bufs=2)[:, :, :]
                hc = cpc // 2   # idx columns per half-subtile
                for slot in range(4):
                    wps = psW.tile([128, 2, 512], F32, tag="wps", name="wps", bufs=2)
                    for chalf in range(2):
                        nc.tensor.matmul(
                            wps[:, chalf, 0:hc*16],
                            sels[4*k + slot][:],
                            w4vc[:, c0 + hc*chalf: c0 + hc*(chalf+1), :],
                            start=True, stop=True)
                    nc.scalar.activation(
                        w4r[:, slot, :].rearrange("p (h n) -> p h n", h=2),
                        wps[:, 0:2, 0:hc*16],
                        AF.Copy)
                prod = gpool.tile([128, 4, CH], BF16, tag="prod", name="prod", bufs=1)
                nc.vector.tensor_tensor(prod[:, :, :], g[:, :, :], w4r[:, :, :], op=mult)
                nc.vector.tensor_tensor(prod[:, 0:2, :], prod[:, 0:2, :], prod[:, 2:4, :], op=add_)
                xq = gpool.tile([128, CH], BF16, tag="xqk", name="xqk", bufs=2)[:, :]
                nc.vector.tensor_tensor(xq[:, :], prod[:, 0, :], prod[:, 1, :], op=add_)
                for ot in range(2):
                    for nt in range(2):
                        if nt * NH >= CH:
                            continue
                        nc.tensor.matmul(
                            yps[ot][nt][:, 0:min(NH, CH - nt*NH)],
                            w_conv_t[k][:, ot*128:(ot+1)*128],
                            xq[:, nt*NH:min((nt+1)*NH, CH)],
                            start=(k == 0), stop=(k == NPT - 1))
            hc2 = cpc // 2
            for ot in range(2):
                yob = gpool.tile([128, 16, cpc], F32, tag=f"yo{ot}", name=f"yo{ot}", bufs=1)[:, :, :]
                for nt in range(2):
                    if nt * NH >= CH:
                        continue
                    nc.scalar.activation(
                        yob[:, :, hc2*nt:hc2*(nt+1)].rearrange("p q c -> p c q"),
                        yps[ot][nt][:, 0:min(NH, CH - nt*NH)], AF.Silu,
                        bias=bnbias[:, ot:ot+1], scale=bnscale[:, ot:ot+1])
                eng = nc.sync if ot == 0 else nc.scalar
                eng.dma_start(
                    out_ap[ot*128:(ot+1)*128, :].rearrange("p (q c) -> p q c", q=16)[:, :, c0:c0+cpc],
                    yob[:, :, :])
    return nc


def make_inputs_per_core(x_img, w_off, b_off, w_conv, bn_gamma, bn_beta, bn_mean, bn_var):
    w_offT = np.ascontiguousarray(
        np.asarray(w_off).transpose(2, 3, 1, 0).reshape(1152, 18)).astype(np.float32)
    w_convT = np.ascontiguousarray(np.asarray(w_conv).T).astype(np.float32)
    bn8 = np.zeros((128, 8), np.float32)
    for i, v in enumerate([bn_gamma, bn_beta, bn_mean, bn_var]):
        v = np.asarray(v)
        bn8[:, 2*i] = v[:128]
        bn8[:, 2*i+1] = v[128:]
    return {
        "x": np.ascontiguousarray(np.asarray(x_img).reshape(C, HW)).astype(np.float32),
        "w_offT": w_offT,
        "b_off": np.asarray(b_off).reshape(18, 1).astype(np.float32),
        "w_convT": w_convT,
        "bn": bn8,
    }


# ----------------------------------------------------------------------
# Harness entry point: full inputs in, full output out.
# Sharding: data-parallel over batch — image b -> NeuronCore b (B=8).
# ----------------------------------------------------------------------
_CACHED = {}

def _get_nc():
    if "nc" not in _CACHED:
        nc = bacc.Bacc("TRN2", target_bir_lowering=False, debug=False,
                       num_devices=8, num_swdge_queues=4)
        build(nc)
        nc.compile()
        _CACHED["nc"] = nc
    return _CACHED["nc"]


def kernel(x, w_off, b_off, w_conv, bn_gamma, bn_beta, bn_mean, bn_var):
    from concourse.bass_utils import run_bass_kernel_spmd
    x = np.asarray(x); w_off = np.asarray(w_off); b_off = np.asarray(b_off)
    w_conv = np.asarray(w_conv)
    bn_gamma = np.asarray(bn_gamma); bn_beta = np.asarray(bn_beta)
    bn_mean = np.asarray(bn_mean); bn_var = np.asarray(bn_var)
    nc = _get_nc()
    in_maps = [make_inputs_per_core(x[c], w_off, b_off, w_conv,
                                    bn_gamma, bn_beta, bn_mean, bn_var)
               for c in range(8)]
    res = run_bass_kernel_spmd(nc, in_maps, core_ids=list(range(8)))
    out = np.stack([res.results[c]["out"].reshape(OUTC, H, W) for c in range(8)])
    return out.astype(np.float32)

